# revision 42
# baseline (speedup 1.0000x reference)
"""Trainium2 Bass kernel for the CompressibleLoss3 pairwise-logdet loss.

Math: for seg = X[cols] with k rows (k=20 for a pair, k=10 per image),
    logdet(I_256 + c * seg^T seg) = logdet(I_k + c * seg seg^T)
(Weinstein-Aronszajn), so each sampled pair needs only a 20x20 Gram matrix
whose 10x10 diagonal blocks also give the per-image logdets.  Scaling is
folded into host-side constants: logdet(I + c G) = k*ln(c) + logdet(G +
(1/c) I), so the device factors M' = G + eps*I and returns sum(ln pivots).

Distribution: 500 pairs padded to 512 = 8 cores x 64; X replicated as bf16.
Per core the 1280 needed rows arrive via indirect-gather (10 groups of 128
rows, chunked so PE transposes start early); DVE copies each transposed
group into a [128, 2x1280] segT image, and 128 accumulating 20-column
matmuls write the 64 20x20 Grams into 4 PSUM tiles.  Act copies them to a
compact [20, 1280] SBUF image whose DRAM bounce (4 narrow writes + 2 reads
interleaved on one queue so DMA inits overlap) reassembles the batch as
AB[pair-partition, 20x20] f32.

Batched LDL^T with pairs on partitions: A20 is eliminated 10 columns with a
"strip" update (only columns j+1..9 are ever re-read; the rows 10..19 x
cols 10..19 Schur block S is updated separately on Pool), leaving S's LDL
pivots = A20's remaining pivots.  The two 10x10 image blocks (A10a/A10b,
stride 210) run as a 2-batched chain on Pool concurrent with the DVE strip
chain; the final S chain runs on DVE.  Ln+accum is split three ways so only
the S-pivot Ln sits on the critical tail.

Toolchain note: walrus accepts only ONE embedded semaphore wait per
instruction; _legalize_waits splits extras into standalone EventSemaphore
instructions (skipped under CoreSim).
"""

import math
import os

import numpy as np

# ---- problem constants (hardcoded; kernel.py must be self-contained) ----
NUM_AUG = 10
EPS = 0.01
GAM3 = 0.01
NUM_PAIRS = 500
M_ROWS, N_FEAT = 4000, 256

N_CORES = 8
B = 64                      # pairs per core (LDL batch, on partitions)
TOTAL_SLOTS = B * N_CORES   # 512 >= 500
K20, K10 = 20, 10
NIDX = B * K20              # 1280 gathered rows per core
N_GROUPS = 10               # gather groups of 128 rows
G_CHUNK = int(os.environ.get("K_GCHUNK", "10"))  # groups per gather instr
N_GATHERS = N_GROUPS // G_CHUNK
TILE_P = 16                 # pairs per PSUM gram tile
N_TILES = B // TILE_P       # 4

C20 = float(N_FEAT / ((2.0 * NUM_AUG + 1e-8) * EPS))
C10 = float(N_FEAT / ((1.0 * NUM_AUG + 1e-8) * EPS))
E20 = 1.0 / C20            # diagonal shift for M' = G + (1/c) I
E10 = 1.0 / C10
LNC20 = K20 * math.log(C20)   # host-side logdet constants
LNC10 = K20 * math.log(C10)   # 2 images x 10 pivots

# AB tile layout (per-partition free offsets, f32 elements)
# A20 at 0 (20x20, row pitch 20); S = A20[10:,10:] at base 210;
# A10a at 420, A10b at 630 (10x10, row pitch 20, batch stride 210)
OFF10A = 420
A10S = 420 if os.environ.get("K_REC3", "1") != "0" else 210
ABW = 1040 if A10S == 420 else 830

_CACHE = {}


def _build_program(loop_n=1):
    import concourse.bass as bass
    import concourse.mybir as mybir
    import concourse.tile as tile
    from concourse.masks import make_identity
    from concourse.tile import add_dep_helper

    f32 = mybir.dt.float32
    bf16 = mybir.dt.bfloat16
    i32 = mybir.dt.int32
    AP = bass.AP
    mult = mybir.AluOpType.mult
    add = mybir.AluOpType.add
    subtract = mybir.AluOpType.subtract
    Ln = mybir.ActivationFunctionType.Ln

    nc = bass.Bass("TRN2", target_bir_lowering=False, debug=False,
                   num_devices=N_CORES)
    X_d = nc.dram_tensor("X", [M_ROWS, N_FEAT], bf16, kind="ExternalInput")
    idx_d = nc.dram_tensor("idx", [128, N_GROUPS], i32, kind="ExternalInput")
    out_d = nc.dram_tensor("out", [B, 3], f32, kind="ExternalOutput")

    def flat(t_ap, off, dims):
        """Raw strided AP over a tile's flat [partitions x pitch] space."""
        return AP(t_ap.tensor, t_ap.offset + off, dims)

    def pitch(t_ap):
        return t_ap.ap[0][0]

    MERGE_CFG = os.environ.get("K_MERGE", "1") != "0"
    with tile.TileContext(nc) as tc:
        with (
            tc.tile_pool(name="const", bufs=1) as constp,
            tc.tile_pool(name="seg", bufs=2) as segp,
            tc.tile_pool(name="ps_t", bufs=1 if MERGE_CFG else 3,
                         space="PSUM") as pst,
            tc.tile_pool(name="ps_g", bufs=1, space="PSUM") as psg,
            tc.tile_pool(name="work", bufs=4) as workp,
            tc.tile_pool(name="dstage", bufs=2, space="DRAM") as dstp,
        ):
            ident = constp.tile([128, 128], bf16, name="ident")
            make_identity(nc, ident[:, :])
            zeros = constp.tile([B, 1], f32, name="zeros")
            nc.vector.memset(zeros[:, :], 0.0)
            warmt = constp.tile([1, 1], f32, name="warmt")
            e20t = constp.tile([B, 1], f32, name="e20t")
            nc.vector.memset(e20t[:, :], E20)
            e10t = constp.tile([B, 1], f32, name="e10t")
            nc.vector.memset(e10t[:, :], E10)
            # warm the Act table (Copy/Ln share natural_log) under the gather
            nc.scalar.copy(warmt[:, :], zeros[0:1, 0:1])
            idx_sb = constp.tile([128, N_GROUPS], i32, name="idx_sb")
            nc.sync.dma_start(idx_sb[:, :], idx_d.ap()[:, :])
            junk_d = dstp.tile([1, 2], f32, name="junk", tag="junk")

            # dummy PE consumer of ident: absorbs the gpsimd-compute wait so
            # later transposes carry only their gather-DMA wait
            if MERGE_CFG:
                tp0 = pst.tile([128, 20 * 128], bf16, name="tp0", tag="tpa")
            else:
                tp0 = pst.tile([128, 2 * 128], bf16, name="tp0", tag="tp")
            nc.tensor.transpose(tp0[:, :128], ident[:, :], ident[:, :])

            PHASE = os.environ.get("K_PHASE", "all")

            def body():
                AB = workp.tile([B, ABW], f32, name="AB", tag="AB")
                ab, apb = AB[:, :], pitch(AB[:, :])
                dstage = dstp.tile([K20, NIDX], f32, name="dstage",
                                   tag="dstage")

                if PHASE in ("all", "front", "dma"):
                    front(AB, ab, apb, dstage)
                if PHASE in ("all", "ldl"):
                    back(AB, ab, apb)

            def front(AB, ab, apb, dstage):
                DO_COMPUTE = PHASE != "dma"
                # --- indirect gather: seg[p, g*256+f] = X[idx[p,g], f] ---
                seg = segp.tile([128, N_GROUPS * N_FEAT], bf16, name="seg",
                                tag="seg")
                sega = seg[:, :]
                for c in range(N_GATHERS):
                    nc.gpsimd.indirect_dma_start(
                        out=seg[:, c * G_CHUNK * N_FEAT:
                                (c + 1) * G_CHUNK * N_FEAT],
                        out_offset=None,
                        in_=X_d.ap(),
                        in_offset=bass.IndirectOffsetOnAxis(
                            ap=idx_sb[:, c * G_CHUNK:(c + 1) * G_CHUNK],
                            axis=0),
                    )

                # --- transpose to segT[f%128, h*1280 + 128g + p] ---
                segT = segp.tile([128, 2 * NIDX], bf16, name="segT",
                                 tag="segT")
                sT = segT[:, :]
                sTp = pitch(sT)
                MERGE = os.environ.get("K_MERGE", "1") != "0"
                if MERGE and DO_COMPUTE:
                    # all 20 transposes into ONE PSUM tile (each 256B out,
                    # bank-aligned), then ONE Act copy into segT
                    tpa = pst.tile([128, 20 * 128], bf16, name="tpa",
                                   tag="tpa")
                    for g in range(N_GROUPS):
                        for h in range(2):
                            nc.tensor.transpose(
                                tpa[:, (2 * g + h) * 128:
                                    (2 * g + h + 1) * 128],
                                seg[:, g * N_FEAT + h * 128:
                                    g * N_FEAT + (h + 1) * 128],
                                ident[:, :])
                    tpf = tpa[:, :]
                    nc.scalar.copy(
                        flat(sT, 0,
                             [[sTp, 128], [128, N_GROUPS],
                              [NIDX, 2], [1, 128]]),
                        flat(tpf, 0,
                             [[pitch(tpf), 128], [256, N_GROUPS],
                              [128, 2], [1, 128]]))
                for g2 in range(N_GROUPS // 2
                                if (DO_COMPUTE and not MERGE) else 0):
                    # two groups (4 transposes) per PSUM tile -> one copy
                    tp = pst.tile([128, 4 * 128], bf16, name="tp", tag="tp")
                    for k in range(2):
                        g = 2 * g2 + k
                        for h in range(2):
                            nc.tensor.transpose(
                                tp[:, (2 * k + h) * 128:
                                   (2 * k + h + 1) * 128],
                                seg[:, g * N_FEAT + h * 128:
                                    g * N_FEAT + (h + 1) * 128],
                                ident[:, :])
                    # tp layout: [k, h, p]; dest segT[h-plane, 128(2g2+k)+p]
                    tcopy = os.environ.get("K_TCOPY", "act")
                    use_act = (tcopy == "act" or
                               (tcopy == "mix" and g2 % 2 == 1))
                    dst_ap = flat(sT, 128 * 2 * g2,
                                  [[sTp, 128], [128, 2], [NIDX, 2], [1, 128]])
                    if use_act:
                        nc.scalar.copy(dst_ap, tp[:, :])
                    else:
                        nc.vector.tensor_scalar(
                            out=dst_ap, in0=tp[:, :],
                            scalar1=0.0, scalar2=None, op0=add)

                # --- 64 20x20 Grams: 2 accumulating matmuls per pair ---
                gcs = []
                if MERGE and DO_COMPUTE:
                    # one padded PSUM tile: pair p at 512*(p//16)+20*(p%16)
                    # (512 f32 = one 2KB bank; matmuls never cross banks)
                    gca = psg.tile([K20, 4 * 512], f32, name="gca",
                                   tag="gca")
                    for p in range(B):
                        off = 512 * (p // TILE_P) + K20 * (p % TILE_P)
                        for h in range(2):
                            op = flat(sT, h * NIDX + K20 * p,
                                      [[sTp, 128], [1, K20]])
                            nc.tensor.matmul(
                                gca[0:K20, off:off + K20],
                                lhsT=op, rhs=op,
                                start=(h == 0), stop=(h == 1))
                for t in range(N_TILES if (DO_COMPUTE and not MERGE) else 0):
                    gc = psg.tile([K20, TILE_P * K20], f32, name=f"gc{t}",
                                  tag=f"gc{t}")
                    gcs.append(gc)
                    for q in range(TILE_P):
                        p = t * TILE_P + q
                        for h in range(2):
                            op = flat(sT, h * NIDX + K20 * p,
                                      [[sTp, 128], [1, K20]])
                            nc.tensor.matmul(
                                gc[0:K20, K20 * q:K20 * (q + 1)],
                                lhsT=op, rhs=op,
                                start=(h == 0), stop=(h == 1))

                # --- compact [20, 1280] f32 image (Act), DRAM bounce on SP:
                # 4 narrow writes + 2 reads, interleaved so inits pipeline ---
                gs = segp.tile([K20, NIDX], f32, name="gs", tag="gs")
                W = TILE_P * K20  # 320
                if MERGE and DO_COMPUTE:
                    # one compacting copy: strips the 192-f32 bank padding
                    gf = gca[:, :]
                    nc.scalar.copy(
                        flat(gs[:, :], 0,
                             [[pitch(gs[:, :]), K20], [W, 4], [1, W]]),
                        flat(gf, 0,
                             [[pitch(gf), K20], [512, 4], [1, W]]))
                for t in range(N_TILES if (DO_COMPUTE and not MERGE) else 0):
                    nc.scalar.copy(gs[:, t * W:(t + 1) * W], gcs[t][:, :])
                if not DO_COMPUTE:
                    # keep the DMA chain ordered without compute
                    nc.vector.tensor_scalar(
                        out=gs[0:K20, 0:1], in0=seg[0:K20, 0:1],
                        scalar1=0.0, scalar2=None, op0=add)
                dsa = dstage[:, :]

                def wdma(t):
                    nc.sync.dma_start(dstage[:, t * W:(t + 1) * W],
                                      gs[:, t * W:(t + 1) * W])

                def rdma(c):
                    # pair p's 20x20 -> AB partition p; src free offset for
                    # (p, r, c) = r*1280 + 20p + c
                    src = AP(dsa.tensor, dsa.offset + K20 * (B // 2) * c,
                             [[K20, B // 2], [NIDX, K20], [1, K20]])
                    dst = AP(ab.tensor, ab.offset + apb * (B // 2) * c,
                             [[apb, B // 2], [K20, K20], [1, K20]])
                    nc.sync.dma_start(dst, src)

                wmode = os.environ.get("K_BOUNCE", "w1r1")
                if wmode == "w4r2":
                    wdma(0); wdma(1); rdma(0); wdma(2); wdma(3); rdma(1)
                elif wmode == "w2r2":
                    nc.sync.dma_start(dstage[:, 0:2 * W], gs[:, 0:2 * W])
                    rdma(0)
                    nc.sync.dma_start(dstage[:, 2 * W:], gs[:, 2 * W:])
                    rdma(1)
                elif wmode == "w1r1":
                    nc.sync.dma_start(dstage[:, :], gs[:, :])
                    src1 = AP(dsa.tensor, dsa.offset,
                              [[K20, B], [NIDX, K20], [1, K20]])
                    dst1 = AP(ab.tensor, ab.offset,
                              [[apb, B], [K20, K20], [1, K20]])
                    nc.sync.dma_start(dst1, src1)
                else:  # w2r1
                    nc.sync.dma_start(dstage[:, 0:2 * W], gs[:, 0:2 * W])
                    nc.sync.dma_start(dstage[:, 2 * W:], gs[:, 2 * W:])
                    src1 = AP(dsa.tensor, dsa.offset,
                              [[K20, B], [NIDX, K20], [1, K20]])
                    dst1 = AP(ab.tensor, ab.offset,
                              [[apb, B], [K20, K20], [1, K20]])
                    nc.sync.dma_start(dst1, src1)

            def back(AB, ab, apb):
                # --- A10s = diagonal 10x10 blocks of A20; copies and
                # diagonal shifts on Act (K_MISC=act) or Pool/DVE ---
                if os.environ.get("K_MISC", "pool") == "act":
                    nc.scalar.copy(
                        flat(ab, OFF10A,
                             [[apb, B], [A10S, 2], [K20, K10], [1, K10]]),
                        flat(ab, 0,
                             [[apb, B], [210, 2], [K20, K10], [1, K10]]))
                    nc.scalar.add(
                        flat(ab, 0, [[apb, B], [21, K20]]),
                        flat(ab, 0, [[apb, B], [21, K20]]), e20t[:, 0:1])
                    nc.scalar.add(
                        flat(ab, OFF10A, [[apb, B], [A10S, 2], [21, K10]]),
                        flat(ab, OFF10A, [[apb, B], [A10S, 2], [21, K10]]),
                        e10t[:, 0:1])
                else:
                    nc.gpsimd.tensor_scalar(
                        out=flat(ab, OFF10A,
                                 [[apb, B], [A10S, 2], [K20, K10], [1, K10]]),
                        in0=flat(ab, 0,
                                 [[apb, B], [210, 2], [K20, K10], [1, K10]]),
                        scalar1=0.0, scalar2=None, op0=add)
                    nc.vector.tensor_scalar(
                        out=flat(ab, 0, [[apb, B], [21, K20]]),
                        in0=flat(ab, 0, [[apb, B], [21, K20]]),
                        scalar1=E20, scalar2=None, op0=add)
                    nc.gpsimd.tensor_scalar(
                        out=flat(ab, OFF10A,
                                 [[apb, B], [A10S, 2], [21, K10]]),
                        in0=flat(ab, OFF10A,
                                 [[apb, B], [A10S, 2], [21, K10]]),
                        scalar1=E10, scalar2=None, op0=add)

                # --- batched LDL^T ---
                LDL = os.environ.get("K_LDL", "strip")
                SQSUB = os.environ.get("K_SQSUB", "pool")
                P = workp.tile([B, 361], f32, name="P", tag="P")
                PS = workp.tile([B, 100], f32, name="PS", tag="PS")
                w3 = workp.tile([B, 3 * (K10 - 1)], f32, name="w3", tag="w3")
                P3 = workp.tile([B, 3 * (K10 - 1) * (K10 - 1)], f32,
                                name="P3", tag="P3")
                invh = workp.tile([B, 1], f32, name="invh", tag="invh")
                invs = workp.tile([B, 1], f32, name="invs", tag="invs")
                invd2 = workp.tile([B, 3], f32, name="invd2", tag="invd2")
                aP, apP = P[:, :], pitch(P[:, :])
                aPS, apPS = PS[:, :], pitch(PS[:, :])
                aw, apw = w3[:, :], pitch(w3[:, :])
                aP2, apP2 = P3[:, :], pitch(P3[:, :])
                lnt = workp.tile([B, K20], f32, name="lnt", tag="lnt")
                osb = workp.tile([B, 3], f32, name="osb", tag="osb")
                lf, apL = lnt[:, :], pitch(lnt[:, :])
                a10_last = [None]

                A10E = os.environ.get("K_A10", "dve")

                def a10_step(j, nmat=2):
                    # nmat-batch chain step j over (S,)A10a,A10b
                    eng = nc.gpsimd if A10E == "pool" else nc.vector
                    st = A10S if nmat == 2 else 210
                    m = K10 - 1 - j
                    base = OFF10A if nmat == 2 else 210
                    col1 = base + (j + 1) * 20 + j
                    if not (A10S == 420 and nmat == 2):
                        nc.vector.reciprocal(
                            invd2[:, 1:1 + nmat] if nmat == 2
                            else invd2[:, 0:nmat],
                            flat(ab, base + j * 21, [[apb, B], [st, nmat]]))
                    eng.tensor_tensor(
                        out=flat(aw, 0, [[apw, B], [K10 - 1, nmat], [1, m]]),
                        in0=flat(ab, col1, [[apb, B], [st, nmat], [20, m]]),
                        in1=AP(invd2[:, :].tensor,
                               invd2[:, :].offset + (1 if nmat == 2 else 0),
                               [[pitch(invd2[:, :]), B], [1, nmat], [0, m]]),
                        op=mult)
                    eng.tensor_tensor(
                        out=flat(aP2, 0, [[apP2, B], [81, nmat],
                                          [m, m], [1, m]]),
                        in0=flat(aw, 0, [[apw, B], [K10 - 1, nmat],
                                         [1, m], [0, m]]),
                        in1=flat(ab, col1, [[apb, B], [st, nmat],
                                            [0, m], [20, m]]),
                        op=mult)
                    a10_last[0] = eng.tensor_tensor(
                        out=flat(ab, base + (j + 1) * 21,
                                 [[apb, B], [st, nmat], [20, m], [1, m]]),
                        in0=flat(ab, base + (j + 1) * 21,
                                 [[apb, B], [st, nmat], [20, m], [1, m]]),
                        in1=flat(aP2, 0, [[apP2, B], [81, nmat],
                                          [m, m], [1, m]]),
                        op=subtract)

                # Phase A: eliminate A20 cols 0..9.
                for j in range(K10):
                    if A10S == 420:
                        # one reciprocal covers A20 + A10a + A10b diagonals
                        nc.vector.reciprocal(
                            invd2[:, 0:3],
                            flat(ab, j * 21, [[apb, B], [420, 3]]))
                        nc.vector.tensor_scalar(
                            out=invh[:, 0:1], in0=invd2[:, 0:1],
                            scalar1=0.0, scalar2=None, op0=add) \
                            if False else None
                    else:
                        nc.vector.reciprocal(
                            invh[:, 0:1],
                            flat(ab, j * 21, [[apb, B], [1, 1]]))
                    n = 19 - j
                    w = 9 - j
                    colb = (j + 1) * 20 + j
                    if LDL == "strip":
                        # strip (cols j+1..9) on DVE; S block on Pool;
                        # concurrent 2-batch A10 chain on Pool
                        if w > 0:
                            nc.vector.scalar_tensor_tensor(
                                out=flat(aP, 0, [[apP, B], [w, n], [1, w]]),
                                in0=flat(ab, colb,
                                         [[apb, B], [20, n], [0, w]]),
                                scalar=invd2[:, 0:1] if A10S == 420 else invh[:, 0:1],
                                in1=flat(ab, colb,
                                         [[apb, B], [0, n], [20, w]]),
                                op0=mult, op1=mult)
                            aeng = (nc.gpsimd if os.environ.get(
                                "K_ASUB", "dve") == "pool" else nc.vector)
                            aeng.tensor_tensor(
                                out=flat(ab, (j + 1) * 21,
                                         [[apb, B], [20, n], [1, w]]),
                                in0=flat(ab, (j + 1) * 21,
                                         [[apb, B], [20, n], [1, w]]),
                                in1=flat(aP, 0, [[apP, B], [w, n], [1, w]]),
                                op=subtract)
                        nc.vector.scalar_tensor_tensor(
                            out=flat(aPS, 0,
                                     [[apPS, B], [K10, K10], [1, K10]]),
                            in0=flat(ab, 200 + j,
                                     [[apb, B], [20, K10], [0, K10]]),
                            scalar=invd2[:, 0:1] if A10S == 420
                            else invh[:, 0:1],
                            in1=flat(ab, 200 + j,
                                     [[apb, B], [0, K10], [20, K10]]),
                            op0=mult, op1=mult)
                        seng = (nc.gpsimd
                                if os.environ.get("K_SSUB", "dve") == "pool"
                                else nc.vector)
                        seng.tensor_tensor(
                            out=flat(ab, 210,
                                     [[apb, B], [20, K10], [1, K10]]),
                            in0=flat(ab, 210,
                                     [[apb, B], [20, K10], [1, K10]]),
                            in1=flat(aPS, 0,
                                     [[apPS, B], [K10, K10], [1, K10]]),
                            op=subtract)
                    else:
                        # full-square trailing update (covers S directly)
                        nc.vector.scalar_tensor_tensor(
                            out=flat(aP, 0, [[apP, B], [n, n], [1, n]]),
                            in0=flat(ab, colb, [[apb, B], [20, n], [0, n]]),
                            scalar=invh[:, 0:1],
                            in1=flat(ab, colb, [[apb, B], [0, n], [20, n]]),
                            op0=mult, op1=mult)
                        sub_eng = (nc.gpsimd if SQSUB == "pool"
                                   else nc.vector)
                        sub_eng.tensor_tensor(
                            out=flat(ab, (j + 1) * 21,
                                     [[apb, B], [20, n], [1, n]]),
                            in0=flat(ab, (j + 1) * 21,
                                     [[apb, B], [20, n], [1, n]]),
                            in1=flat(aP, 0, [[apP, B], [n, n], [1, n]]),
                            op=subtract)
                    if LDL != "base3" and j < K10 - 1:
                        a10_step(j, nmat=2)

                # Ln of A20 pivots 0..9 + Ln of A10 pivots: off the tail
                LN2 = os.environ.get("K_LN2", "1") != "0"
                if not LN2:
                    nc.scalar.activation(
                        out=lnt[:, 0:K10],
                        in_=flat(ab, 0, [[apb, B], [21, K10]]),
                        func=Ln, bias=zeros[:, 0:1], accum_out=osb[:, 2:3])

                # Phase B
                if LDL == "base3":
                    for j in range(K10 - 1):
                        a10_step(j, nmat=3)
                else:
                    for j in range(K10 - 1):
                        m = K10 - 1 - j
                        colb = 210 + (j + 1) * 20 + j
                        nc.vector.reciprocal(
                            invs[:, 0:1],
                            flat(ab, 210 + j * 21, [[apb, B], [1, 1]]))
                        nc.vector.scalar_tensor_tensor(
                            out=flat(aPS, 0, [[apPS, B], [m, m], [1, m]]),
                            in0=flat(ab, colb, [[apb, B], [20, m], [0, m]]),
                            scalar=invs[:, 0:1],
                            in1=flat(ab, colb, [[apb, B], [0, m], [20, m]]),
                            op0=mult, op1=mult)
                        beng = (nc.gpsimd if os.environ.get(
                            "K_BSUB", "dve") == "pool" else nc.vector)
                        beng.tensor_tensor(
                            out=flat(ab, 210 + (j + 1) * 21,
                                     [[apb, B], [20, m], [1, m]]),
                            in0=flat(ab, 210 + (j + 1) * 21,
                                     [[apb, B], [20, m], [1, m]]),
                            in1=flat(aPS, 0, [[apPS, B], [m, m], [1, m]]),
                            op=subtract)

                nc.scalar.activation(
                    out=flat(lf, 0, [[apL, B], [K10, 2], [1, K10]]),
                    in_=flat(ab, OFF10A, [[apb, B], [A10S, 2], [21, K10]]),
                    func=Ln, bias=zeros[:, 0:1], accum_out=osb[:, 1:2])
                # S-pivot Ln (tail); with K_LN2 one Ln covers all 20
                if LN2:
                    nc.scalar.activation(
                        out=lnt[:, 0:K20],
                        in_=flat(ab, 0, [[apb, B], [21, K20]]),
                        func=Ln, bias=zeros[:, 0:1], accum_out=osb[:, 0:1])
                else:
                    nc.scalar.activation(
                        out=lnt[:, K10:K20],
                        in_=flat(ab, 210, [[apb, B], [21, K10]]),
                        func=Ln, bias=zeros[:, 0:1], accum_out=osb[:, 0:1])

                # dummy DMA keyed off the A10 chain: keeps the SP DMA queue
                # hot so the final out-DMA's init overlaps the S chain
                if os.environ.get("K_TAIL", "0") != "0":
                    dj = nc.sync.dma_start(junk_d[:, :], osb[0:1, 1:3])
                    if a10_last[0] is not None:
                        add_dep_helper(dj.ins, a10_last[0].ins,
                                       reason="tail warm after a10 chain")
                if LN2:
                    nc.sync.dma_start(out_d.ap()[:, 0:2], osb[:, 0:2])
                else:
                    nc.sync.dma_start(out_d.ap()[:, :], osb[:, 0:3])

            for _ in range(loop_n):
                body()

    mybir.codegen_inst_isa_subclasses(nc)
    if not os.environ.get("K_SIM"):
        _legalize_waits(nc, mybir)
    return nc


def _legalize_waits(nc, mybir):
    """Split multi-wait instructions into standalone single-wait
    EventSemaphore instructions (this toolchain's codegen allows only one
    embedded semaphore wait per instruction)."""
    n_split = 0
    for f in nc.m.functions:
        for blk in f.blocks:
            insts = blk.instructions
            k = 0
            while k < len(insts):
                ins = insts[k]
                si = ins.sync_info
                if si is not None and si.on_wait and len(si.on_wait) > 1:
                    waits = list(si.on_wait)
                    for m, w in enumerate(waits[:-1]):
                        ev = mybir.InstEventSemaphore(
                            name=f"{ins.name}-lw{m}", engine=ins.engine,
                            sync_info=mybir.SyncInfo(on_wait=[w],
                                                     on_update=[]))
                        insts.insert(k, ev)
                        k += 1
                    si.on_wait = [waits[-1]]
                    n_split += 1
                k += 1
    return n_split


def _get_program():
    if "nc" not in _CACHE:
        loop_n = int(os.environ.get("K_LOOP", "1"))
        _CACHE["nc"] = _build_program(loop_n=loop_n)
    return _CACHE["nc"]


def _make_in_maps(X, sample_pairs):
    import ml_dtypes
    Xc = np.ascontiguousarray(
        np.asarray(X, dtype=np.float32).astype(ml_dtypes.bfloat16))
    sp = np.asarray(sample_pairs, dtype=np.int64)
    padded = np.concatenate(
        [sp, np.broadcast_to(sp[:1], (TOTAL_SLOTS - sp.shape[0], 2))], axis=0)
    aug = np.arange(NUM_AUG, dtype=np.int64)
    in_maps = []
    for c in range(N_CORES):
        pc = padded[c * B:(c + 1) * B]                      # [64, 2]
        cols_i = pc[:, 0:1] * NUM_AUG + aug                 # [64, 10]
        cols_j = pc[:, 1:2] * NUM_AUG + aug                 # [64, 10]
        rows = np.concatenate([cols_i, cols_j], axis=1)     # [64, 20]
        # gather slot k = 20p + r; idx[p2, g] = rows_flat[128g + p2]
        idx = rows.reshape(-1).reshape(N_GROUPS, 128).T
        in_maps.append({
            "X": Xc,
            "idx": np.ascontiguousarray(idx, dtype=np.int32),
        })
    return in_maps


def _postprocess(per_core_outs):
    lds = np.concatenate(per_core_outs, axis=0)[:NUM_PAIRS].astype(np.float64)
    if os.environ.get("K_LN2", "1") != "0":
        ld_pair = lds[:, 0] + LNC20
    else:
        ld_pair = lds[:, 0] + lds[:, 2] + LNC20
    ld_ij = lds[:, 1] + LNC10          # ld_i + ld_j per pair
    ortho = np.mean(ld_pair - 0.5 * ld_ij)
    discrimn = np.mean(ld_pair)
    compress = np.mean(ld_ij)
    total = GAM3 * -ortho
    return np.array([total, discrimn, compress, ortho], dtype=np.float32)


def run_on_hw(X, sample_pairs, trace=False, **spmd_kwargs):
    from concourse.bass_utils import run_bass_kernel_spmd
    nc = _get_program()
    in_maps = _make_in_maps(X, sample_pairs)
    res = run_bass_kernel_spmd(nc, in_maps, core_ids=list(range(N_CORES)),
                               trace=trace, **spmd_kwargs)
    out = _postprocess([r["out"] for r in res.results])
    return out, res


def kernel(X, y=None, sample_pairs=None):
    out, _ = run_on_hw(X, sample_pairs, trace=False)
    return out
